# revision 5
# baseline (speedup 1.0000x reference)
import sys
import os

sys.path.insert(0, "/opt/trn_rl_repo")

import numpy as np
import ml_dtypes

from concourse import bacc, mybir, tile
from concourse.bass_utils import run_bass_kernel_spmd
from concourse.masks import make_identity

B, T, D, H = 32, 512, 512, 1024
NCORES = 8
HS = H // NCORES          # 128 hidden columns owned per core
G = 4 * HS                # 512 gate columns per core: [f | i | g | o]
KV = H // 128             # 8 K-chunks for the V matmul
KU = D // 128             # 4 K-chunks for the U matmul

F32 = mybir.dt.float32
BF16 = mybir.dt.bfloat16
SIG = mybir.ActivationFunctionType.Sigmoid
TANH = mybir.ActivationFunctionType.Tanh
IDENT = mybir.ActivationFunctionType.Identity

BFNP = ml_dtypes.bfloat16

# gate order in all packed layouts: f, i, g, o
GATE_F, GATE_I, GATE_G, GATE_O = 0, 1, 2, 3


def build(t_steps=T):
    nc = bacc.Bacc("TRN2", target_bir_lowering=False, debug=False, num_devices=NCORES)

    TB = t_steps * B
    NBTC = TB // 512          # phase-1 chunks of 512 bt-columns (16 steps each)

    xT_dr = nc.dram_tensor("xT", [D, TB], BF16, kind="ExternalInput")
    V_dr = nc.dram_tensor("Vsel", [128, KV * G], BF16, kind="ExternalInput")
    U_dr = nc.dram_tensor("Usel", [128, KU * G], BF16, kind="ExternalInput")
    bT_dr = nc.dram_tensor("bT", [128, 4], F32, kind="ExternalInput")
    c0T_dr = nc.dram_tensor("c0T", [128, B], F32, kind="ExternalInput")
    h0T_dr = nc.dram_tensor("h0T", [H, B], BF16, kind="ExternalInput")
    out_dr = nc.dram_tensor("hseqT", [128, TB], F32, kind="ExternalOutput")
    xu_dr = nc.dram_tensor("xu_scr", [128, t_steps, 4 * B], BF16)

    with tile.TileContext(nc) as tc:
        with (
            tc.tile_pool(name="const", bufs=1) as cpool,
            tc.tile_pool(name="xin", bufs=3) as xpool,
            tc.tile_pool(name="xu", bufs=6) as xupool,
            tc.tile_pool(name="ht", bufs=2) as hpool,
            tc.tile_pool(name="work", bufs=2) as wpool,
            tc.tile_pool(name="ps", bufs=2, space="PSUM") as pspool,
            tc.tile_pool(name="psg", bufs=1, space="PSUM") as psgpool,
            tc.tile_pool(name="dram", bufs=2, space="DRAM") as dpool,
        ):
            V_sb = cpool.tile([128, KV * G], BF16)
            U_sb = cpool.tile([128, KU * G], BF16)
            bT_sb = cpool.tile([128, 4], F32)
            c_sb = cpool.tile([128, B], F32)
            ident_f = cpool.tile([128, 128], F32)
            ident_sb = cpool.tile([128, 128], BF16)

            nc.sync.dma_start(V_sb[:], V_dr[:])
            nc.sync.dma_start(U_sb[:], U_dr[:])
            nc.sync.dma_start(bT_sb[:], bT_dr[:])
            nc.sync.dma_start(c_sb[:], c0T_dr[:])
            make_identity(nc, ident_f[:])
            nc.vector.tensor_copy(ident_sb[:], ident_f[:])

            # ---- Phase 1: xu[t] = x_t @ U + b for all t (transposed layout) ----
            for btc in range(NBTC):
                xT_sb = xpool.tile([128, KU * 512], BF16, name="xT_sb")
                nc.sync.dma_start(
                    xT_sb[:].rearrange("p (c n) -> p c n", c=KU),
                    xT_dr[:, btc * 512:(btc + 1) * 512].rearrange(
                        "(c p) n -> p c n", p=128
                    ),
                )
                for g in range(4):
                    ps = pspool.tile([128, 512], F32, name="ps_ph1")
                    for kc in range(KU):
                        nc.tensor.matmul(
                            ps[:],
                            U_sb[:, kc * G + g * 128: kc * G + (g + 1) * 128],
                            xT_sb[:, kc * 512:(kc + 1) * 512],
                            start=(kc == 0),
                            stop=(kc == KU - 1),
                        )
                    xu_st = wpool.tile([128, 512], BF16, name="xu_st")
                    nc.scalar.activation(
                        xu_st[:], ps[:], IDENT, bias=bT_sb[:, g:g + 1]
                    )
                    nc.scalar.dma_start(
                        xu_dr[:, btc * 16:(btc + 1) * 16, g * B:(g + 1) * B],
                        xu_st[:].rearrange("p (t b) -> p t b", t=16),
                    )

            # ---- Phase 2: recurrent scan ----
            for t in range(t_steps):
                xu_sb = xupool.tile([128, 4 * B], BF16, name="xu_sb")
                nc.sync.dma_start(xu_sb[:], xu_dr[:, t, :])

                hT_sb = hpool.tile([128, KV * B], BF16, name="hT_sb")
                if t == 0:
                    nc.sync.dma_start(
                        hT_sb[:].rearrange("p (c b) -> p c b", c=KV),
                        h0T_dr.rearrange("(c p) b -> p c b", p=128),
                    )
                else:
                    nc.sync.dma_start(
                        hT_sb[:].rearrange("p (c b) -> p c b", c=KV),
                        gath_dr[:].rearrange("(c p) b -> p c b", p=128),
                    )

                acts = []
                for g in range(4):
                    ps = psgpool.tile([128, B], F32, name=f"ps_g{g}")
                    nc.tensor.matmul(
                        ps[:],
                        ident_sb[:],
                        xu_sb[:, g * B:(g + 1) * B],
                        start=True,
                        stop=False,
                    )
                    for kc in range(KV):
                        nc.tensor.matmul(
                            ps[:],
                            V_sb[:, kc * G + g * 128: kc * G + (g + 1) * 128],
                            hT_sb[:, kc * B:(kc + 1) * B],
                            start=False,
                            stop=(kc == KV - 1),
                        )
                    a = wpool.tile([128, B], F32, name=f"act_g{g}")
                    nc.scalar.activation(
                        a[:], ps[:], TANH if g == GATE_G else SIG
                    )
                    acts.append(a)

                f_a, i_a, g_a, o_a = acts
                t1 = wpool.tile([128, B], F32, name="t1")
                nc.vector.tensor_mul(t1[:], f_a[:], c_sb[:])
                t2 = wpool.tile([128, B], F32, name="t2")
                nc.vector.tensor_mul(t2[:], i_a[:], g_a[:])
                nc.vector.tensor_add(c_sb[:], t1[:], t2[:])
                tc_sb = wpool.tile([128, B], F32, name="tc_sb")
                nc.scalar.activation(tc_sb[:], c_sb[:], TANH)

                h_bf = wpool.tile([128, B], BF16, name="h_bf")
                nc.vector.tensor_mul(h_bf[:], o_a[:], tc_sb[:])

                if t < t_steps - 1:
                    send_dr = dpool.tile([HS, B], BF16, name="send_dr")
                    nc.scalar.dma_start(send_dr[:], h_bf[:])
                    gath_dr = dpool.tile(
                        [H, B], BF16, name="gath_dr", addr_space="Shared"
                    )
                    nc.gpsimd.collective_compute(
                        "AllGather",
                        mybir.AluOpType.bypass,
                        replica_groups=[list(range(NCORES))],
                        ins=[send_dr.opt()],
                        outs=[gath_dr.opt()],
                    )

                h_f32 = wpool.tile([128, B], F32, name="h_f32")
                nc.vector.tensor_mul(h_f32[:], o_a[:], tc_sb[:])
                nc.scalar.dma_start(out_dr[:, t * B:(t + 1) * B], h_f32[:])

    nc.compile()
    return nc


_cache = {}
TRACE = False
LAST_EXEC_NS = None
LAST_RESULT = None


def _get_nc(t_steps=T):
    if t_steps not in _cache:
        _cache[t_steps] = build(t_steps)
    return _cache[t_steps]


def _prep(x, h0, c0, U_i, V_i, b_i, U_f, V_f, b_f, U_o, V_o, b_o, U_g, V_g, b_g):
    x = np.asarray(x, dtype=np.float32)
    h0 = np.asarray(h0, dtype=np.float32)
    c0 = np.asarray(c0, dtype=np.float32)
    # gate order f, i, g, o
    Us = [np.asarray(a, np.float32) for a in (U_f, U_i, U_g, U_o)]
    Vs = [np.asarray(a, np.float32) for a in (V_f, V_i, V_g, V_o)]
    bs = [np.asarray(a, np.float32) for a in (b_f, b_i, b_g, b_o)]

    t_steps = x.shape[1]
    TB = t_steps * B
    xT = np.ascontiguousarray(
        x.transpose(2, 1, 0).reshape(D, TB)
    ).astype(BFNP)
    h0T = np.ascontiguousarray(h0.T).astype(BFNP)

    maps = []
    for r in range(NCORES):
        sl = slice(r * HS, (r + 1) * HS)
        Vcat = np.concatenate([V[:, sl] for V in Vs], axis=1)      # [H, G]
        Vsel = np.ascontiguousarray(
            Vcat.reshape(KV, 128, G).transpose(1, 0, 2).reshape(128, KV * G)
        ).astype(BFNP)
        Ucat = np.concatenate([U[:, sl] for U in Us], axis=1)      # [D, G]
        Usel = np.ascontiguousarray(
            Ucat.reshape(KU, 128, G).transpose(1, 0, 2).reshape(128, KU * G)
        ).astype(BFNP)
        bT = np.ascontiguousarray(
            np.stack([b[sl] for b in bs], axis=1)
        ).astype(np.float32)                                       # [128, 4]
        c0T = np.ascontiguousarray(c0[:, sl].T).astype(np.float32)
        maps.append(
            {
                "xT": xT,
                "Vsel": Vsel,
                "Usel": Usel,
                "bT": bT,
                "c0T": c0T,
                "h0T": h0T,
            }
        )
    return maps


def _unshard(results, t_steps=T):
    outs = []
    for r in range(NCORES):
        hseqT = results[r]["hseqT"]                     # [128, T*B]
        outs.append(
            hseqT.reshape(HS, t_steps, B).transpose(2, 1, 0)  # [B, T, HS]
        )
    return np.ascontiguousarray(np.concatenate(outs, axis=2))


def kernel(x, h0, c0, U_i, V_i, b_i, U_f, V_f, b_f, U_o, V_o, b_o, U_g, V_g, b_g):
    global LAST_EXEC_NS, LAST_RESULT
    in_maps = _prep(
        x, h0, c0, U_i, V_i, b_i, U_f, V_f, b_f, U_o, V_o, b_o, U_g, V_g, b_g
    )
    nc = _get_nc(T)
    res = run_bass_kernel_spmd(nc, in_maps, list(range(NCORES)), trace=TRACE)
    LAST_EXEC_NS = res.exec_time_ns
    LAST_RESULT = res
    return _unshard(res.results, T)
